# revision 31
# baseline (speedup 1.0000x reference)
"""Trainium2 Bass kernel for nn_EnhancedStrategySuperposition.

Grouped 16-expert MLP (Linear-SELU-LayerNorm-Linear-SELU-Linear-Tanh) with
softmax gating, B=32768, D=512, H1=128, H2=64, A=64.

Data-parallel over batch across 8 NeuronCores (4096 rows each), weights
replicated.  Transposed orientation (features on partitions, batch in the
free dimension); the host pre-transposes state once.  ~409us/run vs the
701us v1 baseline (both measured via NTFF on core 0).

Key structure (v3, rebuilt from the profiled v1/v2 baselines):
  * All matmul inputs bf16; PSUM accumulation fp32.
  * LayerNorm mean-correction folded into W2 on the host:
    W2c = W2g - colsum(W2g)/H1, so q = h* @ W2c == h* @ W2g - mu*.c
    exactly.  No mu-correction matmul, no bf16 mu copy.
  * rstd/gate broadcasts never ride the Scalar DMA queue (v1 spent 261us
    there with zero overlap against activations): rstd pairs are
    broadcast by PE selector matmuls ([128,128] full mode - selc/rstd
    zero-padded so no PE tiling-mode switch), gates by stride-0 DMAs
    split across the Sync/GpSimd queues.
  * Gates normalized EARLY (gn = expg * approx-recip(denom) with a f32
    [1->16] broadcast DMA), so the gated fold accumulates the final
    answer and the output is one Act copy + DMA, no divide pass.
  * SELU combine is ONE fused custom DVE op (registered at import):
        ht = min(e1 - alpha, relu(ph + b1))
    via the exact identity elu(u) = min(alpha.e^u - alpha, relu(u)) for
    alpha >= 1 - replacing the relu + min + add three-op chain.
  * All 32 LN stat matmuls batched in one 128x32 col-tiled window (mode
    switches drain the PE array), each expert's mu (PSUM rows 32-47,
    col-tile T1) and sumsq (rows 64-79, T2) pair issued back-to-back so
    the two col tiles co-execute.  Gate logits (rows 0-15), denominator
    (row 0) and the stats share one PSUM bank.
  * The softmax denominator matmul contracts over a zero-padded [128]
    expg tile (memset once per physical slot) so it shares the gate
    matmuls' tiling mode.
  * reciprocal_approx_fast (single DVE op, ~18 bits) for 1/denom and
    1/(var+eps); rstd = Act Sqrt of the reciprocal, emitted bf16, and
    issued AFTER the previous tile's last pair block so the sqrt<->exp
    act-table switch-back is paid by the next gate exp, off the
    critical path.
  * ht^2 on GpSimd (all 16 - measured better than any DVE split);
    small consts packed into two DMA blobs (SP DGE setup is ~565ns per
    DMA and was serializing the kernel head).
  * Fine-grained software pipeline: tile t's pair blocks are woven
    BETWEEN tile t+1's expert blocks (P0 E4 E5 P1 E6 E7 ... stats P6 P7
    sqrt), and each pair issues its q matmuls before the rstd-dependent
    broadcast so no engine queue head-of-line blocks across phases.

Math folds (host-side):
  selu(u) = lam*elu_alpha(u); h* = elu(u) (centered near 0 in bf16),
  true h = lam*h*; lam folds into eps (EPS2 = eps/lam^2) and W3
  (w3 = lam*W3).  W2c = gamma*W2 - colsum(gamma*W2)/H1,
  b2f = b2 + beta@W2.
"""

import os
import math
import numpy as np
import ml_dtypes

import concourse.bass as bass
import concourse.tile as tile
import concourse.mybir as mybir
from concourse import bass_isa
from concourse import bacc
from concourse._compat import with_exitstack
from concourse.bass_utils import run_bass_kernel_spmd

import concourse.dve_ops as dve_ops_mod
from concourse.dve_spec import Spec, Src0, Src1, C0, C2, relu as dve_relu, minn, lower
from concourse.dve_uop import DveOpSpec

F32 = mybir.dt.float32
BF16 = mybir.dt.bfloat16
AF = mybir.ActivationFunctionType
OP = mybir.AluOpType

B_TOTAL = 32768
D = 512
K = 16
H1 = 128
H2 = 64
A = 64
NCORES = 8
BS = B_TOTAL // NCORES          # 4096 rows per core
TILE = 512                      # batch rows per inner tile
NT = BS // TILE                 # 8 tiles per core
NPAIR = K // 2

LAM = 1.0507009873554805
ALPHA = 1.6732632423543772
EPS = 1e-5
EPS2 = EPS / (LAM * LAM)

# engine-assignment knobs
HSQ_POOL = int(os.environ.get("BK_HSQ_POOL", "16"))   # experts 0..n-1: ht^2 on GpSimd
RSB_DVE = int(os.environ.get("BK_RSB_DVE", "0"))     # pairs 8-n..7: rb copy on DVE
YT_DVE = os.environ.get("BK_YT_DVE", "0") == "1"     # yt copy on DVE
Y_DIRECT = os.environ.get("BK_YDIRECT", "0") == "1"  # DMA y straight from PSUM

CSTB_COLS = 4 * K + K * K + NPAIR * 128 + 128 + 1
CST_COLS = K + K + NPAIR + NPAIR + NPAIR + 1

_CACHE = {}

LAST_RESULTS = None             # test.py reads exec_time_ns off this


# --------------------------------------------------------------------------
# fused SELU-combine custom DVE op:  out = min(in0 - imm2, relu(in1 + s0))
# --------------------------------------------------------------------------

def _selu_ref(in0, in1, s0, s1, imm2):
    r = np.maximum(
        np.nan_to_num(in1.astype(np.float32) + s0,
                      nan=0.0, posinf=np.inf, neginf=-np.inf), 0.0)
    return np.minimum(in0.astype(np.float32) - imm2, r)


def _register_selu_op():
    name = "SELU_COMBINE_ANT"
    for op in dve_ops_mod.OPS:
        if op.name == name:
            return op
    spec = Spec(body=minn(Src0 - C2, dve_relu(Src1 + C0)), reference=_selu_ref)
    opcode = max(dve_ops_mod._SUB_OPCODE_FOR_NAME.values()) + 1
    assert opcode < 0x20
    shas = {}
    for ver in ("v3", "v4"):
        uops = lower(spec, ver=ver)
        shas[ver] = DveOpSpec(name=name, opcode=opcode, uops=uops,
                              rd1_en=True).sha(ver)
    op = dve_ops_mod.DveOp(name, spec, subdim=False, uops_sha=shas)
    dve_ops_mod.OPS.append(op)
    dve_ops_mod._SUB_OPCODE_FOR_NAME[name] = opcode
    dve_ops_mod.CUSTOM_DVE_SPECS[name] = spec
    return op


SELU_OP = _register_selu_op()


# --------------------------------------------------------------------------
# device program
# --------------------------------------------------------------------------

@with_exitstack
def _kern(ctx, tc, io, y):
    nc = tc.nc

    def mm(out, lhsT, rhs, **kw):
        nc.tensor.matmul(out, lhsT, rhs, **kw)

    cp = ctx.enter_context(tc.tile_pool(name="consts", bufs=1))
    xp = ctx.enter_context(tc.tile_pool(name="x", bufs=3))
    hp = ctx.enter_context(tc.tile_pool(name="h", bufs=2 * K + 2))
    wa = ctx.enter_context(tc.tile_pool(name="wa", bufs=4))
    wb = ctx.enter_context(tc.tile_pool(name="wb", bufs=4))
    bb = ctx.enter_context(tc.tile_pool(name="bb", bufs=2))
    sp = ctx.enter_context(tc.tile_pool(name="st", bufs=2))
    op_ = ctx.enter_context(tc.tile_pool(name="out", bufs=2))
    ppa = ctx.enter_context(tc.tile_pool(name="psa", bufs=2, space="PSUM"))
    ppg = ctx.enter_context(tc.tile_pool(name="psg", bufs=1, space="PSUM"))
    ppc = ctx.enter_context(tc.tile_pool(name="psc", bufs=2, space="PSUM"))
    pbc = ctx.enter_context(tc.tile_pool(name="psb", bufs=2, space="PSUM"))
    ppf = ctx.enter_context(tc.tile_pool(name="psf", bufs=1, space="PSUM"))

    def cload(name, shape, dt=F32, eng=None):
        t = cp.tile(shape, dt, tag=name)
        (eng or nc.sync).dma_start(t[:], io[name][:])
        return t

    def xs_load(t):
        xs = xp.tile([128, 4, TILE], BF16, tag="xt")
        nc.sync.dma_start(xs[:], io["xt"][:, :, t * TILE:(t + 1) * TILE])
        return xs

    # issue the first two input-tile loads before any weight DMA so tile 0's
    # gate/L1 inputs arrive while the big weight transfers stream in behind.
    nrep = NT * _CACHE.get("repeat", 1)
    xs_pre = {t: xs_load(t % NT) for t in range(min(2, nrep))}

    # all small consts packed into two blobs -> two DMAs instead of ~11
    # (each DMA costs ~565ns of SP sequencer time at the head of the run)
    cstb_t = cload("cstb", [128, CSTB_COLS], BF16)
    cst_t = cload("cst", [128, CST_COLS])

    def bview(a, b):
        return cstb_t[:, a:b]

    o = 0
    gw = bview(o, o + 4 * K); o += 4 * K
    zm = bview(o, o + K * K); o += K * K
    selc = bview(o, o + NPAIR * 128); o += NPAIR * 128
    foldc = bview(o, o + 128); o += 128
    onesc = bview(o, o + 1); o += 1
    o = 0
    b1e = cst_t[:, o:o + K]; o += K
    b1a = cst_t[:, o:o + K]; o += K
    b2e = cst_t[:, o:o + NPAIR]; o += NPAIR
    b2a = cst_t[:, o:o + NPAIR]; o += NPAIR
    b3e = cst_t[:, o:o + NPAIR]; o += NPAIR
    gb = cst_t[0:K, o:o + 1]; o += 1
    N1 = 4 * 128                        # w1 columns per expert
    w1 = cp.tile([128, 4 * N1], BF16, tag="w1")  # experts 0-3, SP ring
    for e in range(4):
        nc.sync.dma_start(w1[:, e * N1:(e + 1) * N1],
                          io["w1"][:, e * N1:(e + 1) * N1])
    w1b = cp.tile([128, 12 * N1], BF16, tag="w1b")
    half = 6 * N1
    nc.scalar.dma_start(w1b[:, :half], io["w1b"][:, :half])
    nc.gpsimd.dma_start(w1b[:, half:], io["w1b"][:, half:])
    w2 = cload("w2", [128, K * 128], BF16)
    w3 = cload("w3", [128, NPAIR * 128], BF16, eng=nc.scalar)

    def w1blk(k, c):
        if k < 4:
            return w1[:, (k * 4 + c) * 128:(k * 4 + c + 1) * 128]
        kk = k - 4
        return w1b[:, (kk * 4 + c) * 128:(kk * 4 + c + 1) * 128]

    def gate_block(t, xs, t_first):
        """Gate matmuls + softmax exp.  Allocates tile t's packed PSUM
        stats bank and returns the state."""
        # one psum bank: gate logits rows 0-15 (row 0 later becomes the
        # softmax denominator), mu stats 32-47, sumsq stats 64-79.
        sgt = ppg.tile([128, TILE], F32, tag="sg")
        for c in range(4):
            mm(sgt[0:K, :], gw[:, c * K:(c + 1) * K], xs[:, c, :],
               start=(c == 0), stop=(c == 3), skip_group_check=True)
        expg = sp.tile([128, TILE], BF16, tag="expg")
        if t_first:
            # rows 16-127 zeroed once per physical slot so the full-128
            # contraction denominator matmul (same tiling mode as the gate
            # matmuls, no PE array drain) sums zeros, not garbage
            nc.vector.memset(expg[:, :], 0.0)
        nc.scalar.activation(expg[0:K, :], sgt[0:K, :], AF.Exp, bias=gb[:])
        return dict(t=t, xs=xs, sgt=sgt, expg=expg,
                    ggbs=[], hts=[], hsqs=[], rstd=None)

    def denom_block(st):
        """Softmax denominator matmul (full-128 contraction over the
        zero-padded expg tile) + approx reciprocal + [1->16] f32 broadcast
        DMA.  Issued one expert block in, so expg (Act) is ready and the
        matmul doesn't stall the PE queue."""
        sgt = st["sgt"]
        mm(sgt[0:1, :], onesc[:], st["expg"][:],
           start=True, stop=True, skip_group_check=True)
        rec1 = sp.tile([1, TILE], F32, tag="rec1")
        nc.vector.reciprocal_approx_fast(out=rec1[:], in_=sgt[0:1, :])
        rec1b = sp.tile([K, TILE], F32, tag="rec1b")
        nc.sync.dma_start(
            rec1b[:], rec1[0:1, :].unsqueeze(1).broadcast_to([1, K, TILE]))
        st["rec1b"] = rec1b

    def gate_bcast(st):
        """Normalize gates and issue the pair broadcasts.  Deferred until a
        few expert blocks in, so the gn multiply (which waits on the Pool
        all-reduce) doesn't head-of-line block the DVE queue in front of
        the expert SELU ops."""
        gn = sp.tile([K, TILE], BF16, tag="gn")
        with nc.allow_low_precision(reason="gates are bf16 like v1"):
            nc.vector.tensor_tensor(gn[:], st["expg"][0:K, :], st["rec1b"][:],
                                    OP.mult)
        for j in range(NPAIR):
            # per-pair tag with 2 bufs: the tile-(t+1) broadcast reuses the
            # slot consumed by pk_j of tile t-1 (long done), never blocking
            # the Sync/GpSimd queues on a same-iteration pk
            ggb = bb.tile([128, TILE], BF16, tag=f"gg{j}")
            eng = nc.sync if j % 2 == 0 else nc.gpsimd
            eng.dma_start(
                ggb[:], gn[2 * j:2 * j + 2, :].unsqueeze(1)
                .broadcast_to([2, 64, TILE]))
            st["ggbs"].append(ggb)

    def expert_block(st, k):
        """Layer-1 + fused SELU + ht^2 for expert k of tile t."""
        xs = st["xs"]
        ph = ppa.tile([128, TILE], F32, tag="ph")
        for c in range(4):
            mm(ph[:], w1blk(k, c),
               xs[:, c, :], start=(c == 0), stop=(c == 3))
        e1 = wa.tile([128, TILE], BF16, tag="e1")
        nc.scalar.activation(e1[:], ph[:], AF.Exp, bias=b1e[:, k:k + 1])
        ht = hp.tile([128, TILE], BF16, tag="ht")
        nc.vector._custom_dve(SELU_OP, out=ht[:], in0=e1[:], in1=ph[:],
                              s0=b1a[:, k:k + 1], s1=0.0, imm2=ALPHA)
        st["hts"].append(ht)
        hsq = wa.tile([128, TILE], BF16, tag="hsq", bufs=2 * K + 2)
        if k < HSQ_POOL:
            nc.gpsimd.tensor_tensor(hsq[:], ht[:], ht[:], OP.mult)
        else:
            nc.vector.tensor_tensor(hsq[:], ht[:], ht[:], OP.mult)
        st["hsqs"].append(hsq)

    def stats_block(st):
        """All 32 LN stat matmuls in one batch: a single 128x32-tiling-mode
        window (entering/leaving col-tiled mode drains the PE array, so
        alternating them with full 128x128 matmuls pays a drain per
        matmul), with each expert's mu (col-tile T1, PSUM rows 32-47) and
        sumsq (T2, rows 64-79) pair issued back-to-back so the two col
        tiles co-execute."""
        sgt = st["sgt"]
        for k in range(K):
            mm(sgt[32:32 + K, :], zm[:, k * K:(k + 1) * K], st["hts"][k][:],
               start=(k == 0), stop=(k == K - 1), skip_group_check=True)
            mm(sgt[64:64 + K, :], zm[:, k * K:(k + 1) * K], st["hsqs"][k][:],
               start=(k == 0), stop=(k == K - 1), skip_group_check=True)

    def stats_tail(st):
        """1/(var + eps2): Square on Act (in every act table, no switch),
        (ssq+eps)-mu^2 on DVE, approx reciprocal on DVE.  Frees the packed
        stats PSUM bank for the next tile's gate matmuls."""
        sgt = st["sgt"]
        m2 = sp.tile([K, TILE], F32, tag="m2")
        nc.scalar.activation(m2[:], sgt[32:32 + K, :], AF.Square)
        veps = sp.tile([K, TILE], F32, tag="veps")
        nc.vector.scalar_tensor_tensor(veps[:], sgt[64:64 + K, :], EPS2, m2[:],
                                       OP.add, OP.subtract)
        vr = sp.tile([K, TILE], F32, tag="vr")
        nc.vector.reciprocal_approx_fast(out=vr[:], in_=veps[:])
        st["vr"] = vr

    def stats_sqrt(st, first):
        """rstd = Sqrt(vr) -> bf16.  Issued AFTER the previous tile's last
        pair block, so the act-table switch back to the exp table is paid
        by the next tile's (non-critical) gate exp, not by a pair e2.
        The per-pair [2x64] broadcasts then go out as stride-0 DMAs
        (Sync/GpSimd queues) and land in SBUF a full phase before the z2
        multiplies consume them - removing the PE selector matmul AND the
        Act PSUM->SBUF copy (a two-hop cross-engine dependency) from
        every pair's critical chain."""
        rstd = sp.tile([K, TILE], BF16, tag="rstd")
        with nc.allow_low_precision(reason="rstd is bf16 like v1"):
            nc.scalar.activation(rstd[:], st["vr"][:], AF.Sqrt)
        st["rstd"] = rstd
        st["rsbs"] = []
        for j in range(NPAIR):
            rsb = bb.tile([128, TILE], BF16, tag=f"rs{j}")
            eng = nc.sync if j % 2 == 0 else nc.gpsimd
            eng.dma_start(
                rsb[:], rstd[2 * j:2 * j + 2, :].unsqueeze(1)
                .broadcast_to([2, 64, TILE]))
            st["rsbs"].append(rsb)

    def pair_block(st, j):
        """Layer-2 LN-apply + SELU + layer-3 + gating for pair j of a
        finished tile; j==0 allocates the fold accumulator, j==7 stores y."""
        hts, ggbs = st["hts"], st["ggbs"]
        ka, kb = 2 * j, 2 * j + 1
        if j == 0:
            st["fold"] = ppf.tile([128, TILE], F32, tag="fold", name="fold")
        fold = st["fold"]
        q = ppc.tile([128, TILE], F32, tag="q")
        mm(q[:], w2[:, ka * 128:ka * 128 + 128], hts[ka][:],
           start=True, stop=False, skip_group_check=True)
        mm(q[:], w2[:, kb * 128:kb * 128 + 128], hts[kb][:],
           start=False, stop=True, skip_group_check=True)
        z2 = wb.tile([128, TILE], BF16, tag="z2")
        with nc.allow_low_precision(reason="z2 feeds bf16 selu chain"):
            nc.vector.tensor_tensor(z2[:], q[:], st["rsbs"][j][:], OP.mult)
        e2 = wb.tile([128, TILE], BF16, tag="e2")
        nc.scalar.activation(e2[:], z2[:], AF.Exp, bias=b2e[:, j:j + 1])
        h2 = wb.tile([128, TILE], BF16, tag="h2")
        nc.vector._custom_dve(SELU_OP, out=h2[:], in0=e2[:], in1=z2[:],
                              s0=b2a[:, j:j + 1], s1=0.0, imm2=ALPHA)
        # layer 3: one full-width matmul per pair (block-diagonal weights)
        ep = pbc.tile([128, TILE], F32, tag="sc")
        mm(ep[:], w3[:, j * 128:(j + 1) * 128], h2[:], start=True, stop=True)
        eo = wb.tile([128, TILE], BF16, tag="eo")
        nc.scalar.activation(eo[:], ep[:], AF.Tanh, bias=b3e[:, j:j + 1])
        with nc.allow_low_precision(reason="pk is a bf16 matmul input"):
            pk = wb.tile([128, TILE], BF16, tag="pk")
            nc.vector.tensor_tensor(pk[:], eo[:], ggbs[j][:], OP.mult)
        # gated sum across pairs: exact fp32 accumulation in PSUM via the
        # fold matmul (sums the two 64-row halves at the same time);
        # gates already normalized, so fold IS the output tile.
        mm(fold[:], foldc[:, 0:128], pk[:],
           start=(j == 0), stop=(j == NPAIR - 1))
        if j == NPAIR - 1:
            t = st["t"]
            yt = op_.tile([A, TILE], F32, tag="yt")
            if YT_DVE:
                nc.vector.tensor_scalar(yt[:], fold[0:A, :], 0.0, None, OP.add)
            else:
                nc.scalar.activation(yt[:], fold[0:A, :], AF.Copy)
            nc.sync.dma_start(y[:, t * TILE:(t + 1) * TILE], yt[:])

    # Software pipeline with fine-grained interleave: tile t's pair blocks
    # are woven BETWEEN tile t+1's expert blocks, so every engine queue
    # alternates between the two phases and nothing head-of-line blocks on
    # a cross-phase dependency.  Pattern per iteration (prev = tile t-1):
    #   gate(t) E0..E3 | P0 E4 E5 | P1 E6 E7 | ... | P5 E14 E15 | stats(t)
    #   P6 P7
    pending = None
    for t in range(nrep):
        xs = xs_pre.pop(t, None)
        if xs is None:
            xs = xs_load(t % NT)
        st = gate_block(t % NT, xs, t_first=(t < 2))
        expert_block(st, 0)
        if t + 1 < nrep and (t + 1) not in xs_pre:
            xs_pre[t + 1] = xs_load((t + 1) % NT)
        denom_block(st)
        for k in range(1, 4):
            expert_block(st, k)
        gate_bcast(st)
        for j in range(6):
            if pending is not None:
                pair_block(pending, j)
            expert_block(st, 4 + 2 * j)
            expert_block(st, 5 + 2 * j)
        stats_block(st)
        stats_tail(st)
        if pending is not None:
            pair_block(pending, 6)
            pair_block(pending, 7)
        stats_sqrt(st, first=(t < 2))
        pending = st
    for j in range(NPAIR):
        pair_block(pending, j)


def _build(repeat=1):
    key = ("nc", repeat)
    if key in _CACHE:
        return _CACHE[key]
    _CACHE["repeat"] = repeat
    nc = bacc.Bacc("TRN2", target_bir_lowering=False, debug=False,
                   enable_asserts=True, num_devices=NCORES)

    def din(name, shape, dt=F32):
        return nc.dram_tensor(name, shape, dt, kind="ExternalInput").ap()

    io = {
        "xt": din("xt", [128, 4, BS], BF16),
        "w1": din("w1", [128, 4 * 4 * 128], BF16),
        "w1b": din("w1b", [128, 12 * 4 * 128], BF16),
        "w2": din("w2", [128, K * 128], BF16),
        "w3": din("w3", [128, NPAIR * 128], BF16),
        "cstb": din("cstb", [128, CSTB_COLS], BF16),
        "cst": din("cst", [128, CST_COLS]),
    }
    y = nc.dram_tensor("y", [A, BS], F32, kind="ExternalOutput").ap()

    with tile.TileContext(nc) as tc:
        _kern(tc, io, y)
    nc.compile()
    _CACHE[key] = nc
    return nc


# --------------------------------------------------------------------------
# host-side preprocessing
# --------------------------------------------------------------------------

def _bf(x):
    return np.ascontiguousarray(np.asarray(x, np.float32).astype(ml_dtypes.bfloat16))


def _prep_consts(W1, b1, ln_gamma, ln_beta, W2, b2, W3, b3, gate_W, gate_b):
    f = np.float32
    W1 = np.asarray(W1, f)
    W2 = np.asarray(W2, f)
    W3 = np.asarray(W3, f)
    b1 = np.asarray(b1, f)
    b2 = np.asarray(b2, f)
    b3 = np.asarray(b3, f)
    ln_gamma = np.asarray(ln_gamma, f)
    ln_beta = np.asarray(ln_beta, f)
    gate_W = np.asarray(gate_W, f)
    gate_b = np.asarray(gate_b, f)

    lna = f(math.log(ALPHA))
    w1h = np.ascontiguousarray(
        W1.reshape(K, 4, 128, H1).transpose(2, 0, 1, 3).reshape(128, K * 4 * 128))
    W2g = ln_gamma[:, :, None] * W2                       # [K,H1,H2]
    # fold the LN mean correction into W2: W2c = W2g - colsum(W2g)/H1
    W2c = W2g - W2g.sum(axis=1, keepdims=True) / H1
    w2h = np.zeros((H1, K * 128), f)
    for k in range(K):
        off = 0 if k % 2 == 0 else H2
        w2h[:, k * 128 + off:k * 128 + off + H2] = W2c[k]
    w3l = LAM * W3                                        # [K,H2,A]
    w3h = np.zeros((128, NPAIR * 128), f)
    for j in range(NPAIR):
        w3h[:H2, j * 128:j * 128 + A] = w3l[2 * j]
        w3h[H2:, j * 128 + A:(j + 1) * 128] = w3l[2 * j + 1]
    gwh = np.ascontiguousarray(
        gate_W.reshape(4, 128, K).transpose(1, 0, 2).reshape(128, 4 * K))

    zmh = np.zeros((128, K * K), f)
    for k in range(K):
        zmh[:, k * K + k] = 1.0 / H1

    selh = np.zeros((128, NPAIR * 128), f)
    for j in range(NPAIR):
        selh[2 * j, j * 128:j * 128 + H2] = 1.0
        selh[2 * j + 1, j * 128 + H2:(j + 1) * 128] = 1.0

    b2f = b2 + np.einsum("kh,khm->km", ln_beta, W2)       # [K,H2]

    def pair_pack(v):                                     # [K,64] -> [128,NPAIR]
        out = np.empty((128, NPAIR), f)
        for j in range(NPAIR):
            out[:H2, j] = v[2 * j]
            out[H2:, j] = v[2 * j + 1]
        return out

    w1hb = _bf(w1h)
    foldh = np.hstack([np.vstack([np.eye(A, dtype=f), np.eye(A, dtype=f)]),
                       np.zeros((128, 128 - A), f)])
    cstb = np.hstack([gwh, zmh, selh, foldh, np.ones((128, 1), f)])
    gbp = np.zeros((128, 1), f)
    gbp[:K, 0] = gate_b
    cst = np.hstack([(b1 + lna).T, b1.T,
                     pair_pack(b2f + lna).astype(f),
                     pair_pack(b2f).astype(f),
                     pair_pack(b3).astype(f), gbp])
    return {
        "w1": np.ascontiguousarray(w1hb[:, :4 * 4 * 128]),
        "w1b": np.ascontiguousarray(w1hb[:, 4 * 4 * 128:]),
        "w2": _bf(w2h), "w3": _bf(w3h),
        "cstb": _bf(cstb),
        "cst": np.ascontiguousarray(cst, f),
    }


def _prep_x(state_shard):
    # [BS, 512] -> [128, 4, BS]:  xt[p, c, b] = state[b, c*128 + p]
    a = np.asarray(state_shard, np.float32).T.reshape(4, 128, BS)
    return _bf(a.transpose(1, 0, 2))


def kernel(state, W1, b1, ln_gamma, ln_beta, W2, b2, W3, b3, gate_W, gate_b):
    global LAST_RESULTS
    nc = _build()
    consts = _prep_consts(W1, b1, ln_gamma, ln_beta, W2, b2, W3, b3,
                          gate_W, gate_b)
    state = np.asarray(state, np.float32)
    in_maps = []
    for c in range(NCORES):
        m = dict(consts)
        m["xt"] = _prep_x(state[c * BS:(c + 1) * BS])
        in_maps.append(m)

    trace = os.environ.get("BASS_KERNEL_TRACE") == "1"
    res = run_bass_kernel_spmd(nc, in_maps, core_ids=list(range(NCORES)),
                               trace=trace)
    LAST_RESULTS = res
    out = np.empty((B_TOTAL, A), np.float32)
    for c in range(NCORES):
        out[c * BS:(c + 1) * BS] = res.results[c]["y"].T
    return out


if __name__ == "__main__":
    rng = np.random.default_rng(0)
    ins = {
        "state": rng.standard_normal((B_TOTAL, D)).astype(np.float32),
        "W1": (rng.standard_normal((K, D, H1)) / np.sqrt(D)).astype(np.float32),
        "b1": np.zeros((K, H1), np.float32),
        "ln_gamma": np.ones((K, H1), np.float32),
        "ln_beta": np.zeros((K, H1), np.float32),
        "W2": (rng.standard_normal((K, H1, H2)) / np.sqrt(H1)).astype(np.float32),
        "b2": np.zeros((K, H2), np.float32),
        "W3": (rng.standard_normal((K, H2, A)) / np.sqrt(H2)).astype(np.float32),
        "b3": np.zeros((K, A), np.float32),
        "gate_W": (rng.standard_normal((D, K)) / np.sqrt(D)).astype(np.float32),
        "gate_b": np.zeros((K,), np.float32),
    }
    y = kernel(**ins)
    print("ok", y.shape, y.dtype, float(np.abs(y).max()))


# revision 32
# speedup vs baseline: 1.2224x; 1.2224x over previous
"""Trainium2 Bass kernel for nn_EnhancedStrategySuperposition.

Grouped 16-expert MLP (Linear-SELU-LayerNorm-Linear-SELU-Linear-Tanh) with
softmax gating, B=32768, D=512, H1=128, H2=64, A=64.

Data-parallel over batch across 8 NeuronCores (4096 rows each), weights
replicated.  Transposed orientation (features on partitions, batch in the
free dimension); the host pre-transposes state once.  ~409us/run vs the
701us v1 baseline (both measured via NTFF on core 0).

Key structure (v3, rebuilt from the profiled v1/v2 baselines):
  * All matmul inputs bf16; PSUM accumulation fp32.
  * LayerNorm mean-correction folded into W2 on the host:
    W2c = W2g - colsum(W2g)/H1, so q = h* @ W2c == h* @ W2g - mu*.c
    exactly.  No mu-correction matmul, no bf16 mu copy.
  * rstd/gate broadcasts never ride the Scalar DMA queue (v1 spent 261us
    there with zero overlap against activations): rstd pairs are
    broadcast by PE selector matmuls ([128,128] full mode - selc/rstd
    zero-padded so no PE tiling-mode switch), gates by stride-0 DMAs
    split across the Sync/GpSimd queues.
  * Gates normalized EARLY (gn = expg * approx-recip(denom) with a f32
    [1->16] broadcast DMA), so the gated fold accumulates the final
    answer and the output is one Act copy + DMA, no divide pass.
  * SELU combine is ONE fused custom DVE op (registered at import):
        ht = min(e1 - alpha, relu(ph + b1))
    via the exact identity elu(u) = min(alpha.e^u - alpha, relu(u)) for
    alpha >= 1 - replacing the relu + min + add three-op chain.
  * All 32 LN stat matmuls batched in one 128x32 col-tiled window (mode
    switches drain the PE array), each expert's mu (PSUM rows 32-47,
    col-tile T1) and sumsq (rows 64-79, T2) pair issued back-to-back so
    the two col tiles co-execute.  Gate logits (rows 0-15), denominator
    (row 0) and the stats share one PSUM bank.
  * The softmax denominator matmul contracts over a zero-padded [128]
    expg tile (memset once per physical slot) so it shares the gate
    matmuls' tiling mode.
  * reciprocal_approx_fast (single DVE op, ~18 bits) for 1/denom and
    1/(var+eps); rstd = Act Sqrt of the reciprocal, emitted bf16, and
    issued AFTER the previous tile's last pair block so the sqrt<->exp
    act-table switch-back is paid by the next gate exp, off the
    critical path.
  * ht^2 on GpSimd (all 16 - measured better than any DVE split);
    small consts packed into two DMA blobs (SP DGE setup is ~565ns per
    DMA and was serializing the kernel head).
  * Fine-grained software pipeline: tile t's pair blocks are woven
    BETWEEN tile t+1's expert blocks (P0 E4 E5 P1 E6 E7 ... stats P6 P7
    sqrt), and each pair issues its q matmuls before the rstd-dependent
    broadcast so no engine queue head-of-line blocks across phases.

Math folds (host-side):
  selu(u) = lam*elu_alpha(u); h* = elu(u) (centered near 0 in bf16),
  true h = lam*h*; lam folds into eps (EPS2 = eps/lam^2) and W3
  (w3 = lam*W3).  W2c = gamma*W2 - colsum(gamma*W2)/H1,
  b2f = b2 + beta@W2.
"""

import os
import math
import numpy as np
import ml_dtypes

import concourse.bass as bass
import concourse.tile as tile
import concourse.mybir as mybir
from concourse import bass_isa
from concourse import bacc
from concourse._compat import with_exitstack
from concourse.bass_utils import run_bass_kernel_spmd

import concourse.dve_ops as dve_ops_mod
from concourse.dve_spec import Spec, Src0, Src1, C0, C2, relu as dve_relu, minn, lower
from concourse.dve_uop import DveOpSpec

F32 = mybir.dt.float32
BF16 = mybir.dt.bfloat16
AF = mybir.ActivationFunctionType
OP = mybir.AluOpType

B_TOTAL = 32768
D = 512
K = 16
H1 = 128
H2 = 64
A = 64
NCORES = 8
BS = B_TOTAL // NCORES          # 4096 rows per core
TILE = 512                      # batch rows per inner tile
NT = BS // TILE                 # 8 tiles per core
NPAIR = K // 2

LAM = 1.0507009873554805
ALPHA = 1.6732632423543772
EPS = 1e-5
EPS2 = EPS / (LAM * LAM)

# engine-assignment knobs
HSQ_POOL = int(os.environ.get("BK_HSQ_POOL", "16"))   # experts 0..n-1: ht^2 on GpSimd
RSB_DVE = int(os.environ.get("BK_RSB_DVE", "0"))     # pairs 8-n..7: rb copy on DVE
YT_DVE = os.environ.get("BK_YT_DVE", "0") == "1"     # yt copy on DVE
Y_DIRECT = os.environ.get("BK_YDIRECT", "0") == "1"  # DMA y straight from PSUM

CSTB_COLS = 4 * K + K * K + NPAIR * 128 + 128 + 1
CST_COLS = K + K + NPAIR + NPAIR + NPAIR + 1

_CACHE = {}

LAST_RESULTS = None             # test.py reads exec_time_ns off this


# --------------------------------------------------------------------------
# fused SELU-combine custom DVE op:  out = min(in0 - imm2, relu(in1 + s0))
# --------------------------------------------------------------------------

def _selu_ref(in0, in1, s0, s1, imm2):
    r = np.maximum(
        np.nan_to_num(in1.astype(np.float32) + s0,
                      nan=0.0, posinf=np.inf, neginf=-np.inf), 0.0)
    return np.minimum(in0.astype(np.float32) - imm2, r)


def _register_selu_op():
    name = "SELU_COMBINE_ANT"
    for op in dve_ops_mod.OPS:
        if op.name == name:
            return op
    spec = Spec(body=minn(Src0 - C2, dve_relu(Src1 + C0)), reference=_selu_ref)
    opcode = max(dve_ops_mod._SUB_OPCODE_FOR_NAME.values()) + 1
    assert opcode < 0x20
    shas = {}
    for ver in ("v3", "v4"):
        uops = lower(spec, ver=ver)
        shas[ver] = DveOpSpec(name=name, opcode=opcode, uops=uops,
                              rd1_en=True).sha(ver)
    op = dve_ops_mod.DveOp(name, spec, subdim=False, uops_sha=shas)
    dve_ops_mod.OPS.append(op)
    dve_ops_mod._SUB_OPCODE_FOR_NAME[name] = opcode
    dve_ops_mod.CUSTOM_DVE_SPECS[name] = spec
    return op


SELU_OP = _register_selu_op()


# --------------------------------------------------------------------------
# device program
# --------------------------------------------------------------------------

@with_exitstack
def _kern(ctx, tc, io, y):
    nc = tc.nc

    def mm(out, lhsT, rhs, **kw):
        nc.tensor.matmul(out, lhsT, rhs, **kw)

    cp = ctx.enter_context(tc.tile_pool(name="consts", bufs=1))
    xp = ctx.enter_context(tc.tile_pool(name="x", bufs=2))
    hp = ctx.enter_context(tc.tile_pool(name="h", bufs=2 * K + 2))
    wa = ctx.enter_context(tc.tile_pool(name="wa", bufs=4))
    wb = ctx.enter_context(tc.tile_pool(name="wb", bufs=4))
    bb = ctx.enter_context(tc.tile_pool(name="bb", bufs=2))
    sp = ctx.enter_context(tc.tile_pool(name="st", bufs=2))
    op_ = ctx.enter_context(tc.tile_pool(name="out", bufs=2))
    ppa = ctx.enter_context(tc.tile_pool(name="psa", bufs=2, space="PSUM"))
    ppg = ctx.enter_context(tc.tile_pool(name="psg", bufs=1, space="PSUM"))
    ppc = ctx.enter_context(tc.tile_pool(name="psc", bufs=2, space="PSUM"))
    pbc = ctx.enter_context(tc.tile_pool(name="psb", bufs=2, space="PSUM"))
    ppf = ctx.enter_context(tc.tile_pool(name="psf", bufs=1, space="PSUM"))

    def cload(name, shape, dt=F32, eng=None):
        t = cp.tile(shape, dt, tag=name)
        (eng or nc.sync).dma_start(t[:], io[name][:])
        return t

    def xs_load(t):
        xs = xp.tile([128, 4, TILE], BF16, tag="xt")
        nc.sync.dma_start(xs[:], io["xt"][:, :, t * TILE:(t + 1) * TILE])
        return xs

    # issue the first two input-tile loads before any weight DMA so tile 0's
    # gate/L1 inputs arrive while the big weight transfers stream in behind.
    nrep = NT * _CACHE.get("repeat", 1)
    xs_pre = {t: xs_load(t % NT) for t in range(min(2, nrep))}

    # all small consts packed into two blobs -> two DMAs instead of ~11
    # (each DMA costs ~565ns of SP sequencer time at the head of the run)
    cstb_t = cload("cstb", [128, CSTB_COLS], BF16)
    cst_t = cload("cst", [128, CST_COLS])

    def bview(a, b):
        return cstb_t[:, a:b]

    o = 0
    gw = bview(o, o + 4 * K); o += 4 * K
    zm = bview(o, o + K * K); o += K * K
    selc = bview(o, o + NPAIR * 128); o += NPAIR * 128
    foldc = bview(o, o + 128); o += 128
    onesc = bview(o, o + 1); o += 1
    o = 0
    b1e = cst_t[:, o:o + K]; o += K
    b1a = cst_t[:, o:o + K]; o += K
    b2e = cst_t[:, o:o + NPAIR]; o += NPAIR
    b2a = cst_t[:, o:o + NPAIR]; o += NPAIR
    b3e = cst_t[:, o:o + NPAIR]; o += NPAIR
    gb = cst_t[0:K, o:o + 1]; o += 1
    N1 = 4 * 128                        # w1 columns per expert
    w1 = cp.tile([128, 4 * N1], BF16, tag="w1")  # experts 0-3, SP ring
    for e in range(4):
        nc.sync.dma_start(w1[:, e * N1:(e + 1) * N1],
                          io["w1"][:, e * N1:(e + 1) * N1])
    w1b = cp.tile([128, 12 * N1], BF16, tag="w1b")
    half = 6 * N1
    nc.scalar.dma_start(w1b[:, :half], io["w1b"][:, :half])
    nc.gpsimd.dma_start(w1b[:, half:], io["w1b"][:, half:])
    w2 = cload("w2", [128, K * 128], BF16)
    w3 = cload("w3", [128, NPAIR * 128], BF16, eng=nc.scalar)

    def w1blk(k, c):
        if k < 4:
            return w1[:, (k * 4 + c) * 128:(k * 4 + c + 1) * 128]
        kk = k - 4
        return w1b[:, (kk * 4 + c) * 128:(kk * 4 + c + 1) * 128]

    def gate_block(t, xs, t_first):
        """Gate matmuls + softmax exp.  Allocates tile t's packed PSUM
        stats bank and returns the state."""
        # one psum bank: gate logits rows 0-15 (row 0 later becomes the
        # softmax denominator), mu stats 32-47, sumsq stats 64-79.
        sgt = ppg.tile([128, TILE], F32, tag="sg")
        for c in range(4):
            mm(sgt[0:K, :], gw[:, c * K:(c + 1) * K], xs[:, c, :],
               start=(c == 0), stop=(c == 3), skip_group_check=True)
        expg = sp.tile([128, TILE], BF16, tag="expg")
        if t_first:
            # rows 16-127 zeroed once per physical slot so the full-128
            # contraction denominator matmul (same tiling mode as the gate
            # matmuls, no PE array drain) sums zeros, not garbage
            nc.vector.memset(expg[:, :], 0.0)
        nc.scalar.activation(expg[0:K, :], sgt[0:K, :], AF.Exp, bias=gb[:])
        return dict(t=t, xs=xs, sgt=sgt, expg=expg,
                    ggbs=[], hts=[], hsqs=[], rstd=None)

    def denom_block(st):
        """Softmax denominator matmul (full-128 contraction over the
        zero-padded expg tile) + approx reciprocal + [1->16] f32 broadcast
        DMA.  Issued one expert block in, so expg (Act) is ready and the
        matmul doesn't stall the PE queue."""
        sgt = st["sgt"]
        mm(sgt[0:1, :], onesc[:], st["expg"][:],
           start=True, stop=True, skip_group_check=True)
        rec1 = sp.tile([1, TILE], F32, tag="rec1")
        nc.vector.reciprocal_approx_fast(out=rec1[:], in_=sgt[0:1, :])
        rec1b = sp.tile([K, TILE], F32, tag="rec1b")
        nc.sync.dma_start(
            rec1b[:], rec1[0:1, :].unsqueeze(1).broadcast_to([1, K, TILE]))
        st["rec1b"] = rec1b

    def gate_bcast(st):
        """Normalize gates and issue the pair broadcasts.  Deferred until a
        few expert blocks in, so the gn multiply (which waits on the Pool
        all-reduce) doesn't head-of-line block the DVE queue in front of
        the expert SELU ops."""
        gn = sp.tile([K, TILE], BF16, tag="gn")
        with nc.allow_low_precision(reason="gates are bf16 like v1"):
            nc.vector.tensor_tensor(gn[:], st["expg"][0:K, :], st["rec1b"][:],
                                    OP.mult)
        for j in range(NPAIR):
            # per-pair tag with 2 bufs: the tile-(t+1) broadcast reuses the
            # slot consumed by pk_j of tile t-1 (long done), never blocking
            # the Sync/GpSimd queues on a same-iteration pk
            ggb = bb.tile([128, TILE], BF16, tag=f"gg{j}")
            eng = nc.sync if j % 2 == 0 else nc.gpsimd
            eng.dma_start(
                ggb[:], gn[2 * j:2 * j + 2, :].unsqueeze(1)
                .broadcast_to([2, 64, TILE]))
            st["ggbs"].append(ggb)

    def expert_block(st, k):
        """Layer-1 + fused SELU + ht^2 for expert k of tile t."""
        xs = st["xs"]
        ph = ppa.tile([128, TILE], F32, tag="ph")
        for c in range(4):
            mm(ph[:], w1blk(k, c),
               xs[:, c, :], start=(c == 0), stop=(c == 3))
        e1 = wa.tile([128, TILE], BF16, tag="e1")
        nc.scalar.activation(e1[:], ph[:], AF.Exp, bias=b1e[:, k:k + 1])
        ht = hp.tile([128, TILE], BF16, tag="ht")
        nc.vector._custom_dve(SELU_OP, out=ht[:], in0=e1[:], in1=ph[:],
                              s0=b1a[:, k:k + 1], s1=0.0, imm2=ALPHA)
        st["hts"].append(ht)
        hsq = wa.tile([128, TILE], BF16, tag="hsq", bufs=2 * K + 2)
        if k < HSQ_POOL:
            nc.gpsimd.tensor_tensor(hsq[:], ht[:], ht[:], OP.mult)
        else:
            nc.vector.tensor_tensor(hsq[:], ht[:], ht[:], OP.mult)
        st["hsqs"].append(hsq)

    def stats_block(st):
        """All 32 LN stat matmuls in one batch: a single 128x32-tiling-mode
        window (entering/leaving col-tiled mode drains the PE array, so
        alternating them with full 128x128 matmuls pays a drain per
        matmul), with each expert's mu (col-tile T1, PSUM rows 32-47) and
        sumsq (T2, rows 64-79) pair issued back-to-back so the two col
        tiles co-execute."""
        sgt = st["sgt"]
        for k in range(K):
            mm(sgt[32:32 + K, :], zm[:, k * K:(k + 1) * K], st["hts"][k][:],
               start=(k == 0), stop=(k == K - 1), skip_group_check=True)
            mm(sgt[64:64 + K, :], zm[:, k * K:(k + 1) * K], st["hsqs"][k][:],
               start=(k == 0), stop=(k == K - 1), skip_group_check=True)

    def stats_tail(st):
        """1/(var + eps2): Square on Act (in every act table, no switch),
        (ssq+eps)-mu^2 on DVE, approx reciprocal on DVE.  Frees the packed
        stats PSUM bank for the next tile's gate matmuls."""
        sgt = st["sgt"]
        m2 = sp.tile([K, TILE], F32, tag="m2")
        nc.scalar.activation(m2[:], sgt[32:32 + K, :], AF.Square)
        veps = sp.tile([K, TILE], F32, tag="veps")
        nc.vector.scalar_tensor_tensor(veps[:], sgt[64:64 + K, :], EPS2, m2[:],
                                       OP.add, OP.subtract)
        vr = sp.tile([K, TILE], F32, tag="vr")
        nc.vector.reciprocal_approx_fast(out=vr[:], in_=veps[:])
        st["vr"] = vr

    def stats_sqrt(st, first):
        """rstd = Sqrt(vr) -> bf16 rows 0-15 of a full-128 tile.  Issued
        AFTER the previous tile's last pair block, so the act-table switch
        back to the exp table is paid by the next tile's (non-critical)
        gate exp, not by a pair e2.  Rows 16-127 are zeroed once per
        physical slot (2-buf rotation) so the full-contraction broadcast
        matmul (which avoids a PE tiling-mode switch) multiplies zeros,
        not garbage."""
        rstd = sp.tile([128, TILE], BF16, tag="rstd")
        if first:
            nc.vector.memset(rstd[:, :], 0.0)
        with nc.allow_low_precision(reason="rstd feeds bf16 bcast matmul"):
            nc.scalar.activation(rstd[0:K, :], st["vr"][:], AF.Sqrt)
        st["rstd"] = rstd

    def pair_block(st, j):
        """Layer-2 LN-apply + SELU + layer-3 + gating for pair j of a
        finished tile; j==0 allocates the fold accumulator, j==7 stores y."""
        hts, rstd, ggbs = st["hts"], st["rstd"], st["ggbs"]
        ka, kb = 2 * j, 2 * j + 1
        if j == 0:
            st["fold"] = ppf.tile([128, TILE], F32, tag="fold", name="fold")
        fold = st["fold"]
        # q first: it depends only on hts, so the PE queue never
        # head-of-line blocks on the rstd chain.
        q = ppc.tile([128, TILE], F32, tag="q")
        mm(q[:], w2[:, ka * 128:ka * 128 + 128], hts[ka][:],
           start=True, stop=False, skip_group_check=True)
        mm(q[:], w2[:, kb * 128:kb * 128 + 128], hts[kb][:],
           start=False, stop=True, skip_group_check=True)
        # rstd pair broadcast on the PE: [rstd_2j x64 ; rstd_2j+1 x64].
        # selc rows 16-127 are zero and rstd rows 16-127 zeroed, so the
        # matmul runs full 128x128 (no tiling-mode switch).
        rb = pbc.tile([128, TILE], F32, tag="sc")
        mm(rb[:], selc[:, j * 128:(j + 1) * 128], rstd[:],
           start=True, stop=True)
        rsb = wb.tile([128, TILE], BF16, tag="rsb")
        if j >= NPAIR - RSB_DVE:
            with nc.allow_low_precision(reason="bf16 copy"):
                nc.vector.tensor_scalar(rsb[:], rb[:], 0.0, None, OP.add)
        else:
            nc.scalar.activation(rsb[:], rb[:], AF.Copy)
        z2 = wb.tile([128, TILE], BF16, tag="z2")
        with nc.allow_low_precision(reason="z2 feeds bf16 selu chain"):
            nc.vector.tensor_tensor(z2[:], q[:], rsb[:], OP.mult)
        e2 = wb.tile([128, TILE], BF16, tag="e2")
        nc.scalar.activation(e2[:], z2[:], AF.Exp, bias=b2e[:, j:j + 1])
        h2 = wb.tile([128, TILE], BF16, tag="h2")
        nc.vector._custom_dve(SELU_OP, out=h2[:], in0=e2[:], in1=z2[:],
                              s0=b2a[:, j:j + 1], s1=0.0, imm2=ALPHA)
        # layer 3: one full-width matmul per pair (block-diagonal weights)
        ep = pbc.tile([128, TILE], F32, tag="sc")
        mm(ep[:], w3[:, j * 128:(j + 1) * 128], h2[:], start=True, stop=True)
        eo = wb.tile([128, TILE], BF16, tag="eo")
        nc.scalar.activation(eo[:], ep[:], AF.Tanh, bias=b3e[:, j:j + 1])
        with nc.allow_low_precision(reason="pk is a bf16 matmul input"):
            pk = wb.tile([128, TILE], BF16, tag="pk")
            nc.vector.tensor_tensor(pk[:], eo[:], ggbs[j][:], OP.mult)
        # gated sum across pairs: exact fp32 accumulation in PSUM via the
        # fold matmul (sums the two 64-row halves at the same time);
        # gates already normalized, so fold IS the output tile.
        mm(fold[:], foldc[:, 0:128], pk[:],
           start=(j == 0), stop=(j == NPAIR - 1))
        if j == NPAIR - 1:
            t = st["t"]
            yt = op_.tile([A, TILE], F32, tag="yt")
            if YT_DVE:
                nc.vector.tensor_scalar(yt[:], fold[0:A, :], 0.0, None, OP.add)
            else:
                nc.scalar.activation(yt[:], fold[0:A, :], AF.Copy)
            nc.sync.dma_start(y[:, t * TILE:(t + 1) * TILE], yt[:])

    # Software pipeline with fine-grained interleave: tile t's pair blocks
    # are woven BETWEEN tile t+1's expert blocks, so every engine queue
    # alternates between the two phases and nothing head-of-line blocks on
    # a cross-phase dependency.  Pattern per iteration (prev = tile t-1):
    #   gate(t) E0..E3 | P0 E4 E5 | P1 E6 E7 | ... | P5 E14 E15 | stats(t)
    #   P6 P7
    pending = None
    for t in range(nrep):
        xs = xs_pre.pop(t, None)
        if xs is None:
            xs = xs_load(t % NT)
        st = gate_block(t % NT, xs, t_first=(t < 2))
        expert_block(st, 0)
        denom_block(st)
        for k in range(1, 4):
            expert_block(st, k)
        gate_bcast(st)
        for j in range(6):
            if pending is not None:
                pair_block(pending, j)
            expert_block(st, 4 + 2 * j)
            expert_block(st, 5 + 2 * j)
        stats_block(st)
        stats_tail(st)
        if pending is not None:
            pair_block(pending, 6)
            pair_block(pending, 7)
        stats_sqrt(st, first=(t < 2))
        pending = st
    for j in range(NPAIR):
        pair_block(pending, j)


def _build(repeat=1):
    key = ("nc", repeat)
    if key in _CACHE:
        return _CACHE[key]
    _CACHE["repeat"] = repeat
    nc = bacc.Bacc("TRN2", target_bir_lowering=False, debug=False,
                   enable_asserts=True, num_devices=NCORES)

    def din(name, shape, dt=F32):
        return nc.dram_tensor(name, shape, dt, kind="ExternalInput").ap()

    io = {
        "xt": din("xt", [128, 4, BS], BF16),
        "w1": din("w1", [128, 4 * 4 * 128], BF16),
        "w1b": din("w1b", [128, 12 * 4 * 128], BF16),
        "w2": din("w2", [128, K * 128], BF16),
        "w3": din("w3", [128, NPAIR * 128], BF16),
        "cstb": din("cstb", [128, CSTB_COLS], BF16),
        "cst": din("cst", [128, CST_COLS]),
    }
    y = nc.dram_tensor("y", [A, BS], F32, kind="ExternalOutput").ap()

    with tile.TileContext(nc) as tc:
        _kern(tc, io, y)
    nc.compile()
    _CACHE[key] = nc
    return nc


# --------------------------------------------------------------------------
# host-side preprocessing
# --------------------------------------------------------------------------

def _bf(x):
    return np.ascontiguousarray(np.asarray(x, np.float32).astype(ml_dtypes.bfloat16))


def _prep_consts(W1, b1, ln_gamma, ln_beta, W2, b2, W3, b3, gate_W, gate_b):
    f = np.float32
    W1 = np.asarray(W1, f)
    W2 = np.asarray(W2, f)
    W3 = np.asarray(W3, f)
    b1 = np.asarray(b1, f)
    b2 = np.asarray(b2, f)
    b3 = np.asarray(b3, f)
    ln_gamma = np.asarray(ln_gamma, f)
    ln_beta = np.asarray(ln_beta, f)
    gate_W = np.asarray(gate_W, f)
    gate_b = np.asarray(gate_b, f)

    lna = f(math.log(ALPHA))
    w1h = np.ascontiguousarray(
        W1.reshape(K, 4, 128, H1).transpose(2, 0, 1, 3).reshape(128, K * 4 * 128))
    W2g = ln_gamma[:, :, None] * W2                       # [K,H1,H2]
    # fold the LN mean correction into W2: W2c = W2g - colsum(W2g)/H1
    W2c = W2g - W2g.sum(axis=1, keepdims=True) / H1
    w2h = np.zeros((H1, K * 128), f)
    for k in range(K):
        off = 0 if k % 2 == 0 else H2
        w2h[:, k * 128 + off:k * 128 + off + H2] = W2c[k]
    w3l = LAM * W3                                        # [K,H2,A]
    w3h = np.zeros((128, NPAIR * 128), f)
    for j in range(NPAIR):
        w3h[:H2, j * 128:j * 128 + A] = w3l[2 * j]
        w3h[H2:, j * 128 + A:(j + 1) * 128] = w3l[2 * j + 1]
    gwh = np.ascontiguousarray(
        gate_W.reshape(4, 128, K).transpose(1, 0, 2).reshape(128, 4 * K))

    zmh = np.zeros((128, K * K), f)
    for k in range(K):
        zmh[:, k * K + k] = 1.0 / H1

    selh = np.zeros((128, NPAIR * 128), f)
    for j in range(NPAIR):
        selh[2 * j, j * 128:j * 128 + H2] = 1.0
        selh[2 * j + 1, j * 128 + H2:(j + 1) * 128] = 1.0

    b2f = b2 + np.einsum("kh,khm->km", ln_beta, W2)       # [K,H2]

    def pair_pack(v):                                     # [K,64] -> [128,NPAIR]
        out = np.empty((128, NPAIR), f)
        for j in range(NPAIR):
            out[:H2, j] = v[2 * j]
            out[H2:, j] = v[2 * j + 1]
        return out

    w1hb = _bf(w1h)
    foldh = np.hstack([np.vstack([np.eye(A, dtype=f), np.eye(A, dtype=f)]),
                       np.zeros((128, 128 - A), f)])
    cstb = np.hstack([gwh, zmh, selh, foldh, np.ones((128, 1), f)])
    gbp = np.zeros((128, 1), f)
    gbp[:K, 0] = gate_b
    cst = np.hstack([(b1 + lna).T, b1.T,
                     pair_pack(b2f + lna).astype(f),
                     pair_pack(b2f).astype(f),
                     pair_pack(b3).astype(f), gbp])
    return {
        "w1": np.ascontiguousarray(w1hb[:, :4 * 4 * 128]),
        "w1b": np.ascontiguousarray(w1hb[:, 4 * 4 * 128:]),
        "w2": _bf(w2h), "w3": _bf(w3h),
        "cstb": _bf(cstb),
        "cst": np.ascontiguousarray(cst, f),
    }


def _prep_x(state_shard):
    # [BS, 512] -> [128, 4, BS]:  xt[p, c, b] = state[b, c*128 + p]
    a = np.asarray(state_shard, np.float32).T.reshape(4, 128, BS)
    return _bf(a.transpose(1, 0, 2))


def kernel(state, W1, b1, ln_gamma, ln_beta, W2, b2, W3, b3, gate_W, gate_b):
    global LAST_RESULTS
    nc = _build()
    consts = _prep_consts(W1, b1, ln_gamma, ln_beta, W2, b2, W3, b3,
                          gate_W, gate_b)
    state = np.asarray(state, np.float32)
    in_maps = []
    for c in range(NCORES):
        m = dict(consts)
        m["xt"] = _prep_x(state[c * BS:(c + 1) * BS])
        in_maps.append(m)

    trace = os.environ.get("BASS_KERNEL_TRACE") == "1"
    res = run_bass_kernel_spmd(nc, in_maps, core_ids=list(range(NCORES)),
                               trace=trace)
    LAST_RESULTS = res
    out = np.empty((B_TOTAL, A), np.float32)
    for c in range(NCORES):
        out[c * BS:(c + 1) * BS] = res.results[c]["y"].T
    return out


if __name__ == "__main__":
    rng = np.random.default_rng(0)
    ins = {
        "state": rng.standard_normal((B_TOTAL, D)).astype(np.float32),
        "W1": (rng.standard_normal((K, D, H1)) / np.sqrt(D)).astype(np.float32),
        "b1": np.zeros((K, H1), np.float32),
        "ln_gamma": np.ones((K, H1), np.float32),
        "ln_beta": np.zeros((K, H1), np.float32),
        "W2": (rng.standard_normal((K, H1, H2)) / np.sqrt(H1)).astype(np.float32),
        "b2": np.zeros((K, H2), np.float32),
        "W3": (rng.standard_normal((K, H2, A)) / np.sqrt(H2)).astype(np.float32),
        "b3": np.zeros((K, A), np.float32),
        "gate_W": (rng.standard_normal((D, K)) / np.sqrt(D)).astype(np.float32),
        "gate_b": np.zeros((K,), np.float32),
    }
    y = kernel(**ins)
    print("ok", y.shape, y.dtype, float(np.abs(y).max()))


# revision 33
# speedup vs baseline: 1.2269x; 1.0036x over previous
"""Trainium2 Bass kernel for nn_EnhancedStrategySuperposition.

Grouped 16-expert MLP (Linear-SELU-LayerNorm-Linear-SELU-Linear-Tanh) with
softmax gating, B=32768, D=512, H1=128, H2=64, A=64.

Data-parallel over batch across 8 NeuronCores (4096 rows each), weights
replicated.  Transposed orientation (features on partitions, batch in the
free dimension); the host pre-transposes state once.  ~409us/run vs the
701us v1 baseline (both measured via NTFF on core 0).

Key structure (v3, rebuilt from the profiled v1/v2 baselines):
  * All matmul inputs bf16; PSUM accumulation fp32.
  * LayerNorm mean-correction folded into W2 on the host:
    W2c = W2g - colsum(W2g)/H1, so q = h* @ W2c == h* @ W2g - mu*.c
    exactly.  No mu-correction matmul, no bf16 mu copy.
  * rstd/gate broadcasts never ride the Scalar DMA queue (v1 spent 261us
    there with zero overlap against activations): rstd pairs are
    broadcast by PE selector matmuls ([128,128] full mode - selc/rstd
    zero-padded so no PE tiling-mode switch), gates by stride-0 DMAs
    split across the Sync/GpSimd queues.
  * Gates normalized EARLY (gn = expg * approx-recip(denom) with a f32
    [1->16] broadcast DMA), so the gated fold accumulates the final
    answer and the output is one Act copy + DMA, no divide pass.
  * SELU combine is ONE fused custom DVE op (registered at import):
        ht = min(e1 - alpha, relu(ph + b1))
    via the exact identity elu(u) = min(alpha.e^u - alpha, relu(u)) for
    alpha >= 1 - replacing the relu + min + add three-op chain.
  * All 32 LN stat matmuls batched in one 128x32 col-tiled window (mode
    switches drain the PE array), each expert's mu (PSUM rows 32-47,
    col-tile T1) and sumsq (rows 64-79, T2) pair issued back-to-back so
    the two col tiles co-execute.  Gate logits (rows 0-15), denominator
    (row 0) and the stats share one PSUM bank.
  * The softmax denominator matmul contracts over a zero-padded [128]
    expg tile (memset once per physical slot) so it shares the gate
    matmuls' tiling mode.
  * reciprocal_approx_fast (single DVE op, ~18 bits) for 1/denom and
    1/(var+eps); rstd = Act Sqrt of the reciprocal, emitted bf16, and
    issued AFTER the previous tile's last pair block so the sqrt<->exp
    act-table switch-back is paid by the next gate exp, off the
    critical path.
  * ht^2 on GpSimd (all 16 - measured better than any DVE split);
    small consts packed into two DMA blobs (SP DGE setup is ~565ns per
    DMA and was serializing the kernel head).
  * Fine-grained software pipeline: tile t's pair blocks are woven
    BETWEEN tile t+1's expert blocks (P0 E4 E5 P1 E6 E7 ... stats P6 P7
    sqrt), and each pair issues its q matmuls before the rstd-dependent
    broadcast so no engine queue head-of-line blocks across phases.

Math folds (host-side):
  selu(u) = lam*elu_alpha(u); h* = elu(u) (centered near 0 in bf16),
  true h = lam*h*; lam folds into eps (EPS2 = eps/lam^2) and W3
  (w3 = lam*W3).  W2c = gamma*W2 - colsum(gamma*W2)/H1,
  b2f = b2 + beta@W2.
"""

import os
import math
import numpy as np
import ml_dtypes

import concourse.bass as bass
import concourse.tile as tile
import concourse.mybir as mybir
from concourse import bass_isa
from concourse import bacc
from concourse._compat import with_exitstack
from concourse.bass_utils import run_bass_kernel_spmd

import concourse.dve_ops as dve_ops_mod
from concourse.dve_spec import Spec, Src0, Src1, C0, C2, relu as dve_relu, minn, lower
from concourse.dve_uop import DveOpSpec

F32 = mybir.dt.float32
BF16 = mybir.dt.bfloat16
AF = mybir.ActivationFunctionType
OP = mybir.AluOpType

B_TOTAL = 32768
D = 512
K = 16
H1 = 128
H2 = 64
A = 64
NCORES = 8
BS = B_TOTAL // NCORES          # 4096 rows per core
TILE = 512                      # batch rows per inner tile
NT = BS // TILE                 # 8 tiles per core
NPAIR = K // 2

LAM = 1.0507009873554805
ALPHA = 1.6732632423543772
EPS = 1e-5
EPS2 = EPS / (LAM * LAM)

# engine-assignment knobs
HSQ_POOL = int(os.environ.get("BK_HSQ_POOL", "16"))   # experts 0..n-1: ht^2 on GpSimd
RSB_DVE = int(os.environ.get("BK_RSB_DVE", "0"))     # pairs 8-n..7: rb copy on DVE
YT_DVE = os.environ.get("BK_YT_DVE", "0") == "1"     # yt copy on DVE
Y_DIRECT = os.environ.get("BK_YDIRECT", "0") == "1"  # DMA y straight from PSUM

CSTB_COLS = 4 * K + K * K + NPAIR * 128 + 128 + 1
CST_COLS = K + K + NPAIR + NPAIR + NPAIR + 1

_CACHE = {}

LAST_RESULTS = None             # test.py reads exec_time_ns off this


# --------------------------------------------------------------------------
# fused SELU-combine custom DVE op:  out = min(in0 - imm2, relu(in1 + s0))
# --------------------------------------------------------------------------

def _selu_ref(in0, in1, s0, s1, imm2):
    r = np.maximum(
        np.nan_to_num(in1.astype(np.float32) + s0,
                      nan=0.0, posinf=np.inf, neginf=-np.inf), 0.0)
    return np.minimum(in0.astype(np.float32) - imm2, r)


def _register_selu_op():
    name = "SELU_COMBINE_ANT"
    for op in dve_ops_mod.OPS:
        if op.name == name:
            return op
    spec = Spec(body=minn(Src0 - C2, dve_relu(Src1 + C0)), reference=_selu_ref)
    opcode = max(dve_ops_mod._SUB_OPCODE_FOR_NAME.values()) + 1
    assert opcode < 0x20
    shas = {}
    for ver in ("v3", "v4"):
        uops = lower(spec, ver=ver)
        shas[ver] = DveOpSpec(name=name, opcode=opcode, uops=uops,
                              rd1_en=True).sha(ver)
    op = dve_ops_mod.DveOp(name, spec, subdim=False, uops_sha=shas)
    dve_ops_mod.OPS.append(op)
    dve_ops_mod._SUB_OPCODE_FOR_NAME[name] = opcode
    dve_ops_mod.CUSTOM_DVE_SPECS[name] = spec
    return op


SELU_OP = _register_selu_op()


# --------------------------------------------------------------------------
# device program
# --------------------------------------------------------------------------

@with_exitstack
def _kern(ctx, tc, io, y):
    nc = tc.nc

    def mm(out, lhsT, rhs, **kw):
        nc.tensor.matmul(out, lhsT, rhs, **kw)

    cp = ctx.enter_context(tc.tile_pool(name="consts", bufs=1))
    xp = ctx.enter_context(tc.tile_pool(name="x", bufs=3))
    hp = ctx.enter_context(tc.tile_pool(name="h", bufs=2 * K + 2))
    wa = ctx.enter_context(tc.tile_pool(name="wa", bufs=4))
    wb = ctx.enter_context(tc.tile_pool(name="wb", bufs=4))
    bb = ctx.enter_context(tc.tile_pool(name="bb", bufs=2))
    sp = ctx.enter_context(tc.tile_pool(name="st", bufs=2))
    op_ = ctx.enter_context(tc.tile_pool(name="out", bufs=2))
    ppa = ctx.enter_context(tc.tile_pool(name="psa", bufs=2, space="PSUM"))
    ppg = ctx.enter_context(tc.tile_pool(name="psg", bufs=1, space="PSUM"))
    ppc = ctx.enter_context(tc.tile_pool(name="psc", bufs=2, space="PSUM"))
    pbc = ctx.enter_context(tc.tile_pool(name="psb", bufs=2, space="PSUM"))
    ppf = ctx.enter_context(tc.tile_pool(name="psf", bufs=1, space="PSUM"))

    def cload(name, shape, dt=F32, eng=None):
        t = cp.tile(shape, dt, tag=name)
        (eng or nc.sync).dma_start(t[:], io[name][:])
        return t

    def xs_load(t):
        xs = xp.tile([128, 4, TILE], BF16, tag="xt")
        nc.sync.dma_start(xs[:], io["xt"][:, :, t * TILE:(t + 1) * TILE])
        return xs

    # issue the first two input-tile loads before any weight DMA so tile 0's
    # gate/L1 inputs arrive while the big weight transfers stream in behind.
    nrep = NT * _CACHE.get("repeat", 1)
    xs_pre = {t: xs_load(t % NT) for t in range(min(2, nrep))}

    # all small consts packed into two blobs -> two DMAs instead of ~11
    # (each DMA costs ~565ns of SP sequencer time at the head of the run)
    cstb_t = cload("cstb", [128, CSTB_COLS], BF16)
    cst_t = cload("cst", [128, CST_COLS])

    def bview(a, b):
        return cstb_t[:, a:b]

    o = 0
    gw = bview(o, o + 4 * K); o += 4 * K
    zm = bview(o, o + K * K); o += K * K
    selc = bview(o, o + NPAIR * 128); o += NPAIR * 128
    foldc = bview(o, o + 128); o += 128
    onesc = bview(o, o + 1); o += 1
    o = 0
    b1e = cst_t[:, o:o + K]; o += K
    b1a = cst_t[:, o:o + K]; o += K
    b2e = cst_t[:, o:o + NPAIR]; o += NPAIR
    b2a = cst_t[:, o:o + NPAIR]; o += NPAIR
    b3e = cst_t[:, o:o + NPAIR]; o += NPAIR
    gb = cst_t[0:K, o:o + 1]; o += 1
    N1 = 4 * 128                        # w1 columns per expert
    w1 = cp.tile([128, 4 * N1], BF16, tag="w1")  # experts 0-3, SP ring
    for e in range(4):
        nc.sync.dma_start(w1[:, e * N1:(e + 1) * N1],
                          io["w1"][:, e * N1:(e + 1) * N1])
    w1b = cp.tile([128, 12 * N1], BF16, tag="w1b")
    half = 6 * N1
    nc.scalar.dma_start(w1b[:, :half], io["w1b"][:, :half])
    nc.gpsimd.dma_start(w1b[:, half:], io["w1b"][:, half:])
    w2 = cload("w2", [128, K * 128], BF16)
    w3 = cload("w3", [128, NPAIR * 128], BF16, eng=nc.scalar)

    def w1blk(k, c):
        if k < 4:
            return w1[:, (k * 4 + c) * 128:(k * 4 + c + 1) * 128]
        kk = k - 4
        return w1b[:, (kk * 4 + c) * 128:(kk * 4 + c + 1) * 128]

    def gate_block(t, xs, t_first):
        """Gate matmuls + softmax exp.  Allocates tile t's packed PSUM
        stats bank and returns the state."""
        # one psum bank: gate logits rows 0-15 (row 0 later becomes the
        # softmax denominator), mu stats 32-47, sumsq stats 64-79.
        sgt = ppg.tile([128, TILE], F32, tag="sg")
        for c in range(4):
            mm(sgt[0:K, :], gw[:, c * K:(c + 1) * K], xs[:, c, :],
               start=(c == 0), stop=(c == 3), skip_group_check=True)
        expg = sp.tile([128, TILE], BF16, tag="expg")
        if t_first:
            # rows 16-127 zeroed once per physical slot so the full-128
            # contraction denominator matmul (same tiling mode as the gate
            # matmuls, no PE array drain) sums zeros, not garbage
            nc.vector.memset(expg[:, :], 0.0)
        nc.scalar.activation(expg[0:K, :], sgt[0:K, :], AF.Exp, bias=gb[:])
        return dict(t=t, xs=xs, sgt=sgt, expg=expg,
                    ggbs=[], hts=[], hsqs=[], rstd=None)

    def denom_block(st):
        """Softmax denominator matmul (full-128 contraction over the
        zero-padded expg tile) + approx reciprocal + [1->16] f32 broadcast
        DMA.  Issued one expert block in, so expg (Act) is ready and the
        matmul doesn't stall the PE queue."""
        sgt = st["sgt"]
        mm(sgt[0:1, :], onesc[:], st["expg"][:],
           start=True, stop=True, skip_group_check=True)
        rec1 = sp.tile([1, TILE], F32, tag="rec1")
        nc.vector.reciprocal_approx_fast(out=rec1[:], in_=sgt[0:1, :])
        rec1b = sp.tile([K, TILE], F32, tag="rec1b")
        nc.sync.dma_start(
            rec1b[:], rec1[0:1, :].unsqueeze(1).broadcast_to([1, K, TILE]))
        st["rec1b"] = rec1b

    def gate_bcast(st):
        """Normalize gates and issue the pair broadcasts.  Deferred until a
        few expert blocks in, so the gn multiply (which waits on the Pool
        all-reduce) doesn't head-of-line block the DVE queue in front of
        the expert SELU ops."""
        gn = sp.tile([K, TILE], BF16, tag="gn")
        with nc.allow_low_precision(reason="gates are bf16 like v1"):
            nc.vector.tensor_tensor(gn[:], st["expg"][0:K, :], st["rec1b"][:],
                                    OP.mult)
        for j in range(NPAIR):
            # per-pair tag with 2 bufs: the tile-(t+1) broadcast reuses the
            # slot consumed by pk_j of tile t-1 (long done), never blocking
            # the Sync/GpSimd queues on a same-iteration pk
            ggb = bb.tile([128, TILE], BF16, tag=f"gg{j}")
            eng = nc.sync if j % 2 == 0 else nc.gpsimd
            eng.dma_start(
                ggb[:], gn[2 * j:2 * j + 2, :].unsqueeze(1)
                .broadcast_to([2, 64, TILE]))
            st["ggbs"].append(ggb)

    def expert_block(st, k):
        """Layer-1 + fused SELU + ht^2 for expert k of tile t."""
        xs = st["xs"]
        ph = ppa.tile([128, TILE], F32, tag="ph")
        for c in range(4):
            mm(ph[:], w1blk(k, c),
               xs[:, c, :], start=(c == 0), stop=(c == 3))
        e1 = wa.tile([128, TILE], BF16, tag="e1")
        nc.scalar.activation(e1[:], ph[:], AF.Exp, bias=b1e[:, k:k + 1])
        ht = hp.tile([128, TILE], BF16, tag="ht")
        nc.vector._custom_dve(SELU_OP, out=ht[:], in0=e1[:], in1=ph[:],
                              s0=b1a[:, k:k + 1], s1=0.0, imm2=ALPHA)
        st["hts"].append(ht)
        hsq = wa.tile([128, TILE], BF16, tag="hsq", bufs=2 * K + 2)
        if k < HSQ_POOL:
            nc.gpsimd.tensor_tensor(hsq[:], ht[:], ht[:], OP.mult)
        else:
            nc.vector.tensor_tensor(hsq[:], ht[:], ht[:], OP.mult)
        st["hsqs"].append(hsq)

    def stats_block(st):
        """All 32 LN stat matmuls in one batch: a single 128x32-tiling-mode
        window (entering/leaving col-tiled mode drains the PE array, so
        alternating them with full 128x128 matmuls pays a drain per
        matmul), with each expert's mu (col-tile T1, PSUM rows 32-47) and
        sumsq (T2, rows 64-79) pair issued back-to-back so the two col
        tiles co-execute."""
        sgt = st["sgt"]
        for k in range(K):
            mm(sgt[32:32 + K, :], zm[:, k * K:(k + 1) * K], st["hts"][k][:],
               start=(k == 0), stop=(k == K - 1), skip_group_check=True)
            mm(sgt[64:64 + K, :], zm[:, k * K:(k + 1) * K], st["hsqs"][k][:],
               start=(k == 0), stop=(k == K - 1), skip_group_check=True)

    def stats_tail(st):
        """1/(var + eps2): Square on Act (in every act table, no switch),
        (ssq+eps)-mu^2 on DVE, approx reciprocal on DVE.  Frees the packed
        stats PSUM bank for the next tile's gate matmuls."""
        sgt = st["sgt"]
        m2 = sp.tile([K, TILE], F32, tag="m2")
        nc.scalar.activation(m2[:], sgt[32:32 + K, :], AF.Square)
        veps = sp.tile([K, TILE], F32, tag="veps")
        nc.vector.scalar_tensor_tensor(veps[:], sgt[64:64 + K, :], EPS2, m2[:],
                                       OP.add, OP.subtract)
        vr = sp.tile([K, TILE], F32, tag="vr")
        nc.vector.reciprocal_approx_fast(out=vr[:], in_=veps[:])
        st["vr"] = vr

    def stats_sqrt(st, first):
        """rstd = Sqrt(vr) -> bf16 rows 0-15 of a full-128 tile.  Issued
        AFTER the previous tile's last pair block, so the act-table switch
        back to the exp table is paid by the next tile's (non-critical)
        gate exp, not by a pair e2.  Rows 16-127 are zeroed once per
        physical slot (2-buf rotation) so the full-contraction broadcast
        matmul (which avoids a PE tiling-mode switch) multiplies zeros,
        not garbage."""
        rstd = sp.tile([128, TILE], BF16, tag="rstd")
        if first:
            nc.vector.memset(rstd[:, :], 0.0)
        with nc.allow_low_precision(reason="rstd feeds bf16 bcast matmul"):
            nc.scalar.activation(rstd[0:K, :], st["vr"][:], AF.Sqrt)
        st["rstd"] = rstd

    def pair_block(st, j):
        """Layer-2 LN-apply + SELU + layer-3 + gating for pair j of a
        finished tile; j==0 allocates the fold accumulator, j==7 stores y."""
        hts, rstd, ggbs = st["hts"], st["rstd"], st["ggbs"]
        ka, kb = 2 * j, 2 * j + 1
        if j == 0:
            st["fold"] = ppf.tile([128, TILE], F32, tag="fold", name="fold")
        fold = st["fold"]
        # q first: it depends only on hts, so the PE queue never
        # head-of-line blocks on the rstd chain.
        q = ppc.tile([128, TILE], F32, tag="q")
        mm(q[:], w2[:, ka * 128:ka * 128 + 128], hts[ka][:],
           start=True, stop=False, skip_group_check=True)
        mm(q[:], w2[:, kb * 128:kb * 128 + 128], hts[kb][:],
           start=False, stop=True, skip_group_check=True)
        # rstd pair broadcast on the PE: [rstd_2j x64 ; rstd_2j+1 x64].
        # selc rows 16-127 are zero and rstd rows 16-127 zeroed, so the
        # matmul runs full 128x128 (no tiling-mode switch).
        rb = pbc.tile([128, TILE], F32, tag="sc")
        mm(rb[:], selc[:, j * 128:(j + 1) * 128], rstd[:],
           start=True, stop=True)
        rsb = wb.tile([128, TILE], BF16, tag="rsb")
        if j >= NPAIR - RSB_DVE:
            with nc.allow_low_precision(reason="bf16 copy"):
                nc.vector.tensor_scalar(rsb[:], rb[:], 0.0, None, OP.add)
        else:
            nc.scalar.activation(rsb[:], rb[:], AF.Copy)
        z2 = wb.tile([128, TILE], BF16, tag="z2")
        with nc.allow_low_precision(reason="z2 feeds bf16 selu chain"):
            nc.vector.tensor_tensor(z2[:], q[:], rsb[:], OP.mult)
        e2 = wb.tile([128, TILE], BF16, tag="e2")
        nc.scalar.activation(e2[:], z2[:], AF.Exp, bias=b2e[:, j:j + 1])
        h2 = wb.tile([128, TILE], BF16, tag="h2")
        nc.vector._custom_dve(SELU_OP, out=h2[:], in0=e2[:], in1=z2[:],
                              s0=b2a[:, j:j + 1], s1=0.0, imm2=ALPHA)
        # layer 3: one full-width matmul per pair (block-diagonal weights)
        ep = pbc.tile([128, TILE], F32, tag="sc")
        mm(ep[:], w3[:, j * 128:(j + 1) * 128], h2[:], start=True, stop=True)
        eo = wb.tile([128, TILE], BF16, tag="eo")
        nc.scalar.activation(eo[:], ep[:], AF.Tanh, bias=b3e[:, j:j + 1])
        with nc.allow_low_precision(reason="pk is a bf16 matmul input"):
            pk = wb.tile([128, TILE], BF16, tag="pk")
            nc.vector.tensor_tensor(pk[:], eo[:], ggbs[j][:], OP.mult)
        # gated sum across pairs: exact fp32 accumulation in PSUM via the
        # fold matmul (sums the two 64-row halves at the same time);
        # gates already normalized, so fold IS the output tile.
        mm(fold[:], foldc[:, 0:128], pk[:],
           start=(j == 0), stop=(j == NPAIR - 1))
        if j == NPAIR - 1:
            t = st["t"]
            yt = op_.tile([A, TILE], F32, tag="yt")
            if YT_DVE:
                nc.vector.tensor_scalar(yt[:], fold[0:A, :], 0.0, None, OP.add)
            else:
                nc.scalar.activation(yt[:], fold[0:A, :], AF.Copy)
            nc.sync.dma_start(y[:, t * TILE:(t + 1) * TILE], yt[:])

    # Software pipeline with fine-grained interleave: tile t's pair blocks
    # are woven BETWEEN tile t+1's expert blocks, so every engine queue
    # alternates between the two phases and nothing head-of-line blocks on
    # a cross-phase dependency.  Pattern per iteration (prev = tile t-1):
    #   gate(t) E0..E3 | P0 E4 E5 | P1 E6 E7 | ... | P5 E14 E15 | stats(t)
    #   P6 P7
    pending = None
    for t in range(nrep):
        xs = xs_pre.pop(t, None)
        if xs is None:
            xs = xs_load(t % NT)
        st = gate_block(t % NT, xs, t_first=(t < 2))
        expert_block(st, 0)
        # prefetch the next tile's input one iteration ahead: the xs DMA
        # otherwise sits right on the tile-start critical path (gate
        # matmuls wait ~1.3us for it every tile)
        if t + 1 < nrep and (t + 1) not in xs_pre:
            xs_pre[t + 1] = xs_load((t + 1) % NT)
        denom_block(st)
        for k in range(1, 4):
            expert_block(st, k)
        gate_bcast(st)
        for j in range(6):
            if pending is not None:
                pair_block(pending, j)
            expert_block(st, 4 + 2 * j)
            expert_block(st, 5 + 2 * j)
        stats_block(st)
        stats_tail(st)
        if pending is not None:
            pair_block(pending, 6)
            pair_block(pending, 7)
        stats_sqrt(st, first=(t < 2))
        pending = st
    for j in range(NPAIR):
        pair_block(pending, j)


def _build(repeat=1):
    key = ("nc", repeat)
    if key in _CACHE:
        return _CACHE[key]
    _CACHE["repeat"] = repeat
    nc = bacc.Bacc("TRN2", target_bir_lowering=False, debug=False,
                   enable_asserts=True, num_devices=NCORES)

    def din(name, shape, dt=F32):
        return nc.dram_tensor(name, shape, dt, kind="ExternalInput").ap()

    io = {
        "xt": din("xt", [128, 4, BS], BF16),
        "w1": din("w1", [128, 4 * 4 * 128], BF16),
        "w1b": din("w1b", [128, 12 * 4 * 128], BF16),
        "w2": din("w2", [128, K * 128], BF16),
        "w3": din("w3", [128, NPAIR * 128], BF16),
        "cstb": din("cstb", [128, CSTB_COLS], BF16),
        "cst": din("cst", [128, CST_COLS]),
    }
    y = nc.dram_tensor("y", [A, BS], F32, kind="ExternalOutput").ap()

    with tile.TileContext(nc) as tc:
        _kern(tc, io, y)
    nc.compile()
    _CACHE[key] = nc
    return nc


# --------------------------------------------------------------------------
# host-side preprocessing
# --------------------------------------------------------------------------

def _bf(x):
    return np.ascontiguousarray(np.asarray(x, np.float32).astype(ml_dtypes.bfloat16))


def _prep_consts(W1, b1, ln_gamma, ln_beta, W2, b2, W3, b3, gate_W, gate_b):
    f = np.float32
    W1 = np.asarray(W1, f)
    W2 = np.asarray(W2, f)
    W3 = np.asarray(W3, f)
    b1 = np.asarray(b1, f)
    b2 = np.asarray(b2, f)
    b3 = np.asarray(b3, f)
    ln_gamma = np.asarray(ln_gamma, f)
    ln_beta = np.asarray(ln_beta, f)
    gate_W = np.asarray(gate_W, f)
    gate_b = np.asarray(gate_b, f)

    lna = f(math.log(ALPHA))
    w1h = np.ascontiguousarray(
        W1.reshape(K, 4, 128, H1).transpose(2, 0, 1, 3).reshape(128, K * 4 * 128))
    W2g = ln_gamma[:, :, None] * W2                       # [K,H1,H2]
    # fold the LN mean correction into W2: W2c = W2g - colsum(W2g)/H1
    W2c = W2g - W2g.sum(axis=1, keepdims=True) / H1
    w2h = np.zeros((H1, K * 128), f)
    for k in range(K):
        off = 0 if k % 2 == 0 else H2
        w2h[:, k * 128 + off:k * 128 + off + H2] = W2c[k]
    w3l = LAM * W3                                        # [K,H2,A]
    w3h = np.zeros((128, NPAIR * 128), f)
    for j in range(NPAIR):
        w3h[:H2, j * 128:j * 128 + A] = w3l[2 * j]
        w3h[H2:, j * 128 + A:(j + 1) * 128] = w3l[2 * j + 1]
    gwh = np.ascontiguousarray(
        gate_W.reshape(4, 128, K).transpose(1, 0, 2).reshape(128, 4 * K))

    zmh = np.zeros((128, K * K), f)
    for k in range(K):
        zmh[:, k * K + k] = 1.0 / H1

    selh = np.zeros((128, NPAIR * 128), f)
    for j in range(NPAIR):
        selh[2 * j, j * 128:j * 128 + H2] = 1.0
        selh[2 * j + 1, j * 128 + H2:(j + 1) * 128] = 1.0

    b2f = b2 + np.einsum("kh,khm->km", ln_beta, W2)       # [K,H2]

    def pair_pack(v):                                     # [K,64] -> [128,NPAIR]
        out = np.empty((128, NPAIR), f)
        for j in range(NPAIR):
            out[:H2, j] = v[2 * j]
            out[H2:, j] = v[2 * j + 1]
        return out

    w1hb = _bf(w1h)
    foldh = np.hstack([np.vstack([np.eye(A, dtype=f), np.eye(A, dtype=f)]),
                       np.zeros((128, 128 - A), f)])
    cstb = np.hstack([gwh, zmh, selh, foldh, np.ones((128, 1), f)])
    gbp = np.zeros((128, 1), f)
    gbp[:K, 0] = gate_b
    cst = np.hstack([(b1 + lna).T, b1.T,
                     pair_pack(b2f + lna).astype(f),
                     pair_pack(b2f).astype(f),
                     pair_pack(b3).astype(f), gbp])
    return {
        "w1": np.ascontiguousarray(w1hb[:, :4 * 4 * 128]),
        "w1b": np.ascontiguousarray(w1hb[:, 4 * 4 * 128:]),
        "w2": _bf(w2h), "w3": _bf(w3h),
        "cstb": _bf(cstb),
        "cst": np.ascontiguousarray(cst, f),
    }


def _prep_x(state_shard):
    # [BS, 512] -> [128, 4, BS]:  xt[p, c, b] = state[b, c*128 + p]
    a = np.asarray(state_shard, np.float32).T.reshape(4, 128, BS)
    return _bf(a.transpose(1, 0, 2))


def kernel(state, W1, b1, ln_gamma, ln_beta, W2, b2, W3, b3, gate_W, gate_b):
    global LAST_RESULTS
    nc = _build()
    consts = _prep_consts(W1, b1, ln_gamma, ln_beta, W2, b2, W3, b3,
                          gate_W, gate_b)
    state = np.asarray(state, np.float32)
    in_maps = []
    for c in range(NCORES):
        m = dict(consts)
        m["xt"] = _prep_x(state[c * BS:(c + 1) * BS])
        in_maps.append(m)

    trace = os.environ.get("BASS_KERNEL_TRACE") == "1"
    res = run_bass_kernel_spmd(nc, in_maps, core_ids=list(range(NCORES)),
                               trace=trace)
    LAST_RESULTS = res
    out = np.empty((B_TOTAL, A), np.float32)
    for c in range(NCORES):
        out[c * BS:(c + 1) * BS] = res.results[c]["y"].T
    return out


if __name__ == "__main__":
    rng = np.random.default_rng(0)
    ins = {
        "state": rng.standard_normal((B_TOTAL, D)).astype(np.float32),
        "W1": (rng.standard_normal((K, D, H1)) / np.sqrt(D)).astype(np.float32),
        "b1": np.zeros((K, H1), np.float32),
        "ln_gamma": np.ones((K, H1), np.float32),
        "ln_beta": np.zeros((K, H1), np.float32),
        "W2": (rng.standard_normal((K, H1, H2)) / np.sqrt(H1)).astype(np.float32),
        "b2": np.zeros((K, H2), np.float32),
        "W3": (rng.standard_normal((K, H2, A)) / np.sqrt(H2)).astype(np.float32),
        "b3": np.zeros((K, A), np.float32),
        "gate_W": (rng.standard_normal((D, K)) / np.sqrt(D)).astype(np.float32),
        "gate_b": np.zeros((K,), np.float32),
    }
    y = kernel(**ins)
    print("ok", y.shape, y.dtype, float(np.abs(y).max()))
